# revision 1
# baseline (speedup 1.0000x reference)
"""CrossNetwork kernel for TRN2, 8-core data-parallel.

Reference computation (per layer i in 0..3):
    s_i = <x_i, w_i>            (per-sample dot, feature dim 1024)
    x_{i+1} = x0 * s_i + b_i + x_i

Algebraic collapse used here: x_i = a_i * x0 + d_i with a_0 = 1, d_0 = 0 and
    d_{i+1} = d_i + b_i                  (sample-independent vectors)
    a_{i+1} = a_i * (1 + u_i) + e_i      (per-sample scalars)
where u_i = <x0, w_i> and e_i = <d_i, w_i> (sample-independent scalars).
Output = a_4 * x0 + d_4.

So per sample we only need the 4 dots u_i against x0, a tiny scalar
recurrence, and one fused multiply-add pass over x0.

Sharding: batch dim (16384) split across 8 cores (2048 rows each);
weight_w / weight_b replicated.
"""

import numpy as np

N_FEAT = 1024
N_LAYER = 4
B_FULL = 16384
N_CORES = 8
B_LOCAL = B_FULL // N_CORES      # 2048
P = 128                          # SBUF partitions
N_TILES = B_LOCAL // P           # 16

_CACHE = {}


def _build_nc():
    import concourse.bass as bass
    import concourse.tile as tile
    from concourse import bacc, mybir

    fp32 = mybir.dt.float32
    Alu = mybir.AluOpType

    nc = bacc.Bacc(target_bir_lowering=False)

    x_d = nc.dram_tensor("x", [B_LOCAL, N_FEAT], fp32, kind="ExternalInput")
    w_d = nc.dram_tensor("weight_w", [N_LAYER, N_FEAT], fp32, kind="ExternalInput")
    b_d = nc.dram_tensor("weight_b", [N_LAYER, N_FEAT], fp32, kind="ExternalInput")
    o_d = nc.dram_tensor("out", [B_LOCAL, N_FEAT], fp32, kind="ExternalOutput")

    with tile.TileContext(nc) as tc:
        with (
            tc.tile_pool(name="const", bufs=1) as cpool,
            tc.tile_pool(name="xbuf", bufs=N_TILES) as xpool,
            tc.tile_pool(name="scratch", bufs=2) as spool,
            tc.tile_pool(name="obuf", bufs=3) as opool,
        ):
            # ---- prep: load weights/biases as flat rows on partition 0 ----
            wcat = cpool.tile([1, N_LAYER * N_FEAT], fp32)   # w0|w1|w2|w3
            bcat = cpool.tile([1, N_LAYER * N_FEAT], fp32)
            for i in range(N_LAYER):
                nc.sync.dma_start(wcat[:, i * N_FEAT:(i + 1) * N_FEAT], w_d[i:i + 1, :])
                nc.sync.dma_start(bcat[:, i * N_FEAT:(i + 1) * N_FEAT], b_d[i:i + 1, :])

            # prefix sums d_2, d_3, d_4 (d_1 = b_0 is a view of bcat)
            dpref = cpool.tile([1, 3 * N_FEAT], fp32)
            d1 = bcat[:, 0:N_FEAT]
            d2 = dpref[:, 0:N_FEAT]
            d3 = dpref[:, N_FEAT:2 * N_FEAT]
            d4 = dpref[:, 2 * N_FEAT:3 * N_FEAT]
            nc.vector.tensor_tensor(d2, d1, bcat[:, N_FEAT:2 * N_FEAT], Alu.add)
            nc.vector.tensor_tensor(d3, d2, bcat[:, 2 * N_FEAT:3 * N_FEAT], Alu.add)
            nc.vector.tensor_tensor(d4, d3, bcat[:, 3 * N_FEAT:4 * N_FEAT], Alu.add)

            # e_i = <d_i, w_i>; e_0 = 0
            e_row = cpool.tile([1, N_LAYER], fp32)
            nc.gpsimd.memset(e_row[:], 0.0)
            escr = cpool.tile([1, N_FEAT], fp32)
            for i, di in ((1, d1), (2, d2), (3, d3)):
                nc.vector.scalar_tensor_tensor(
                    escr[:], di, 0.0, wcat[:, i * N_FEAT:(i + 1) * N_FEAT],
                    Alu.bypass, Alu.mult, accum_out=e_row[:, i:i + 1],
                )

            # broadcast to all 128 partitions
            w_rep = cpool.tile([P, N_LAYER * N_FEAT], fp32)
            d4_rep = cpool.tile([P, N_FEAT], fp32)
            e_rep = cpool.tile([P, N_LAYER], fp32)
            nc.gpsimd.partition_broadcast(w_rep[:], wcat[:])
            nc.gpsimd.partition_broadcast(d4_rep[:], d4)
            nc.gpsimd.partition_broadcast(e_rep[:], e_row[:])

            # ---- phase 1: stream x in, compute u_i = <x0, w_i> per row ----
            u_all = cpool.tile([P, N_LAYER, N_TILES], fp32)
            xts = []
            for t in range(N_TILES):
                xt = xpool.tile([P, N_FEAT], fp32)
                xts.append(xt)
                nc.sync.dma_start(xt[:], x_d[t * P:(t + 1) * P, :])
                for i in range(N_LAYER):
                    scr = spool.tile([P, N_FEAT], fp32)
                    nc.vector.scalar_tensor_tensor(
                        scr[:], xt[:], 0.0, w_rep[:, i * N_FEAT:(i + 1) * N_FEAT],
                        Alu.bypass, Alu.mult, accum_out=u_all[:, i, t:t + 1],
                    )

            # ---- phase 2: scalar recurrence a <- a*(1+u_i) + e_i ----
            a = cpool.tile([P, N_TILES], fp32)
            v = cpool.tile([P, N_TILES], fp32)
            a2 = cpool.tile([P, N_TILES], fp32)
            nc.gpsimd.memset(a[:], 1.0)
            for i in range(N_LAYER):
                nc.vector.tensor_scalar(v[:], u_all[:, i, :], 1.0, None, Alu.add)
                nc.vector.tensor_tensor(a2[:], a[:], v[:], Alu.mult)
                nc.vector.tensor_scalar(a[:], a2[:], e_rep[:, i:i + 1], None, Alu.add)

            # ---- phase 3: out = a * x0 + d4, stream out ----
            for t in range(N_TILES):
                ot = opool.tile([P, N_FEAT], fp32)
                nc.vector.scalar_tensor_tensor(
                    ot[:], xts[t][:], a[:, t:t + 1], d4_rep[:],
                    Alu.mult, Alu.add,
                )
                nc.sync.dma_start(o_d[t * P:(t + 1) * P, :], ot[:])

    nc.compile()
    return nc


def _get_nc():
    if "nc" not in _CACHE:
        _CACHE["nc"] = _build_nc()
    return _CACHE["nc"]


def run(x, weight_w, weight_b, trace=False):
    """Run on 8 cores; returns (out_full, BassKernelResults)."""
    from concourse.bass_utils import run_bass_kernel_spmd

    x = np.ascontiguousarray(np.asarray(x, dtype=np.float32))
    weight_w = np.ascontiguousarray(np.asarray(weight_w, dtype=np.float32))
    weight_b = np.ascontiguousarray(np.asarray(weight_b, dtype=np.float32))
    assert x.shape == (B_FULL, N_FEAT)

    nc = _get_nc()
    in_maps = [
        {
            "x": x[c * B_LOCAL:(c + 1) * B_LOCAL],
            "weight_w": weight_w,
            "weight_b": weight_b,
        }
        for c in range(N_CORES)
    ]
    res = run_bass_kernel_spmd(nc, in_maps, list(range(N_CORES)), trace=trace)
    out = np.concatenate([res.results[c]["out"] for c in range(N_CORES)], axis=0)
    return out, res


def kernel(x, weight_w, weight_b):
    out, _ = run(x, weight_w, weight_b, trace=False)
    return out


# revision 2
# speedup vs baseline: 1.2549x; 1.2549x over previous
"""CrossNetwork kernel for TRN2, 8-core data-parallel.

Reference computation (per layer i in 0..3):
    s_i = <x_i, w_i>            (per-sample dot, feature dim 1024)
    x_{i+1} = x0 * s_i + b_i + x_i

Algebraic collapse used here: x_i = a_i * x0 + d_i with a_0 = 1, d_0 = 0 and
    d_{i+1} = d_i + b_i                  (sample-independent vectors)
    a_{i+1} = a_i * (1 + u_i) + e_i      (per-sample scalars)
where u_i = <x0, w_i> and e_i = <d_i, w_i> (sample-independent scalars).
Output = a_4 * x0 + d_4.

So per sample we only need the 4 dots u_i = <x0, w_i>, a tiny scalar
recurrence, and one fused multiply-add pass over x0.

Engine split (per core, 16 row-tiles of [128, 1024]):
  - PE: transpose each x-tile (8x 128x128 blocks) and matmul xT @ W^T to get
    the 4 dots per row; warmed up with a burst of dummy matmuls.
  - ACT: PSUM->SBUF copies (transposed blocks, U results).
  - DVE: weight prep, the a-recurrence, and the final out = a*x0 + d4 pass.
  - DMA: 8 MiB in + 8 MiB out per core (the roofline).

Sharding: batch dim (16384) split across 8 cores (2048 rows each);
weight_w / weight_b replicated.
"""

import numpy as np

N_FEAT = 1024
N_LAYER = 4
B_FULL = 16384
N_CORES = 8
B_LOCAL = B_FULL // N_CORES      # 2048
P = 128                          # SBUF partitions
N_TILES = B_LOCAL // P           # 16
N_BLK = N_FEAT // P              # 8 feature blocks per tile
N_WARM = 40                      # PE warmup matmuls (~4.4us busy -> HAM warm)

_CACHE = {}


def _build_nc():
    import concourse.bass as bass
    import concourse.tile as tile
    from concourse import bacc, mybir
    from concourse.masks import make_identity

    fp32 = mybir.dt.float32
    bf16 = mybir.dt.bfloat16
    Alu = mybir.AluOpType

    nc = bacc.Bacc(target_bir_lowering=False)

    x_d = nc.dram_tensor("x", [B_LOCAL, N_FEAT], fp32, kind="ExternalInput")
    w_d = nc.dram_tensor("weight_w", [N_LAYER, N_FEAT], fp32, kind="ExternalInput")
    b_d = nc.dram_tensor("weight_b", [N_LAYER, N_FEAT], fp32, kind="ExternalInput")
    o_d = nc.dram_tensor("out", [B_LOCAL, N_FEAT], fp32, kind="ExternalOutput")

    with tile.TileContext(nc) as tc:
        with (
            tc.tile_pool(name="const", bufs=1) as cpool,
            tc.tile_pool(name="xbuf", bufs=N_TILES) as xpool,
            tc.tile_pool(name="xtbuf", bufs=3) as xtpool,
            tc.tile_pool(name="obuf", bufs=3) as opool,
            tc.tile_pool(name="psA", bufs=4, space="PSUM") as psA,
            tc.tile_pool(name="psU", bufs=2, space="PSUM") as psU,
            tc.tile_pool(name="psW", bufs=1, space="PSUM") as psW,
        ):
            # ---- identities for PE transpose ----
            ident = cpool.tile([P, P], fp32)
            make_identity(nc, ident[:])
            ident_bf = cpool.tile([P, P], bf16)
            make_identity(nc, ident_bf[:])

            # ---- PE warmup: dense bf16 matmuls to flip HAM to full clock ----
            warm_ps = psW.tile([P, P], fp32)
            for _ in range(N_WARM):
                nc.tensor.matmul(warm_ps[:], ident_bf[:], ident_bf[:])

            # ---- prep: weights/biases ----
            wrows = cpool.tile([N_LAYER, N_FEAT], fp32)
            nc.sync.dma_start(wrows[:], w_d[:])
            wcat = cpool.tile([1, N_LAYER * N_FEAT], fp32)   # w0|w1|w2|w3
            bcat = cpool.tile([1, N_LAYER * N_FEAT], fp32)
            for i in range(N_LAYER):
                nc.sync.dma_start(wcat[:, i * N_FEAT:(i + 1) * N_FEAT], w_d[i:i + 1, :])
                nc.sync.dma_start(bcat[:, i * N_FEAT:(i + 1) * N_FEAT], b_d[i:i + 1, :])

            # W^T blocks: [4, 1024] -> 8 blocks of [128, 4] via PE transpose
            wt_ps = psW.tile([P, N_BLK * N_LAYER], fp32)
            for f in range(N_BLK):
                nc.tensor.matmul(
                    wt_ps[:, f * N_LAYER:(f + 1) * N_LAYER],
                    wrows[:, f * P:(f + 1) * P],
                    ident[:N_LAYER, :N_LAYER],
                    is_transpose=True,
                )
            wt_sb = cpool.tile([P, N_BLK * N_LAYER], fp32)
            nc.scalar.copy(wt_sb[:], wt_ps[:])

            # prefix sums d_2, d_3, d_4 (d_1 = b_0 is a view of bcat)
            dpref = cpool.tile([1, 3 * N_FEAT], fp32)
            d1 = bcat[:, 0:N_FEAT]
            d2 = dpref[:, 0:N_FEAT]
            d3 = dpref[:, N_FEAT:2 * N_FEAT]
            d4 = dpref[:, 2 * N_FEAT:3 * N_FEAT]
            nc.vector.tensor_tensor(d2, d1, bcat[:, N_FEAT:2 * N_FEAT], Alu.add)
            nc.vector.tensor_tensor(d3, d2, bcat[:, 2 * N_FEAT:3 * N_FEAT], Alu.add)
            nc.vector.tensor_tensor(d4, d3, bcat[:, 3 * N_FEAT:4 * N_FEAT], Alu.add)

            # e_i = <d_i, w_i>; e_0 = 0
            e_row = cpool.tile([1, N_LAYER], fp32)
            nc.gpsimd.memset(e_row[:], 0.0)
            escr = cpool.tile([1, N_FEAT], fp32)
            for i, di in ((1, d1), (2, d2), (3, d3)):
                nc.vector.scalar_tensor_tensor(
                    escr[:], di, 0.0, wcat[:, i * N_FEAT:(i + 1) * N_FEAT],
                    Alu.bypass, Alu.mult, accum_out=e_row[:, i:i + 1],
                )

            # broadcast d4 / e to all 128 partitions
            d4_rep = cpool.tile([P, N_FEAT], fp32)
            e_rep = cpool.tile([P, N_LAYER], fp32)
            nc.gpsimd.partition_broadcast(d4_rep[:], d4)
            nc.gpsimd.partition_broadcast(e_rep[:], e_row[:])

            # ---- phase 1: stream x in; PE computes u = x @ W^T per tile ----
            u_all = cpool.tile([P, N_TILES, N_LAYER], fp32)
            xts = []
            for t in range(N_TILES):
                xt = xpool.tile([P, N_FEAT], fp32)
                xts.append(xt)
                nc.sync.dma_start(xt[:], x_d[t * P:(t + 1) * P, :])

                # transpose 8 blocks -> PSUM (2 banks of 4 blocks), copy to SBUF
                xt_sb = xtpool.tile([P, N_FEAT], fp32)
                for h in range(2):
                    tp = psA.tile([P, 4 * P], fp32)
                    for k in range(4):
                        f = h * 4 + k
                        nc.tensor.matmul(
                            tp[:, k * P:(k + 1) * P],
                            xt[:, f * P:(f + 1) * P],
                            ident[:],
                            is_transpose=True,
                        )
                    nc.scalar.copy(xt_sb[:, h * 4 * P:(h + 1) * 4 * P], tp[:])

                # U_t = xT.T @ W^T accumulated over feature blocks -> [128, 4]
                u_ps = psU.tile([P, N_LAYER], fp32)
                for f in range(N_BLK):
                    nc.tensor.matmul(
                        u_ps[:],
                        xt_sb[:, f * P:(f + 1) * P],
                        wt_sb[:, f * N_LAYER:(f + 1) * N_LAYER],
                        start=(f == 0),
                        stop=(f == N_BLK - 1),
                    )
                nc.scalar.copy(u_all[:, t, :], u_ps[:])

            # ---- phase 2: recurrence a <- a*(1+u_i) + e_i over layers ----
            a = cpool.tile([P, N_TILES], fp32)
            v = cpool.tile([P, N_TILES], fp32)
            a2 = cpool.tile([P, N_TILES], fp32)
            nc.gpsimd.memset(a[:], 1.0)
            for i in range(N_LAYER):
                nc.vector.tensor_scalar(v[:], u_all[:, :, i], 1.0, None, Alu.add)
                nc.vector.tensor_tensor(a2[:], a[:], v[:], Alu.mult)
                nc.vector.tensor_scalar(a[:], a2[:], e_rep[:, i:i + 1], None, Alu.add)

            # ---- phase 3: out = a * x0 + d4, stream out ----
            for t in range(N_TILES):
                ot = opool.tile([P, N_FEAT], fp32)
                nc.vector.scalar_tensor_tensor(
                    ot[:], xts[t][:], a[:, t:t + 1], d4_rep[:],
                    Alu.mult, Alu.add,
                )
                nc.sync.dma_start(o_d[t * P:(t + 1) * P, :], ot[:])

    nc.compile()
    return nc


def _get_nc():
    if "nc" not in _CACHE:
        _CACHE["nc"] = _build_nc()
    return _CACHE["nc"]


def run(x, weight_w, weight_b, trace=False):
    """Run on 8 cores; returns (out_full, BassKernelResults)."""
    from concourse.bass_utils import run_bass_kernel_spmd

    x = np.ascontiguousarray(np.asarray(x, dtype=np.float32))
    weight_w = np.ascontiguousarray(np.asarray(weight_w, dtype=np.float32))
    weight_b = np.ascontiguousarray(np.asarray(weight_b, dtype=np.float32))
    assert x.shape == (B_FULL, N_FEAT)

    nc = _get_nc()
    in_maps = [
        {
            "x": x[c * B_LOCAL:(c + 1) * B_LOCAL],
            "weight_w": weight_w,
            "weight_b": weight_b,
        }
        for c in range(N_CORES)
    ]
    res = run_bass_kernel_spmd(nc, in_maps, list(range(N_CORES)), trace=trace)
    out = np.concatenate([res.results[c]["out"] for c in range(N_CORES)], axis=0)
    return out, res


def kernel(x, weight_w, weight_b):
    out, _ = run(x, weight_w, weight_b, trace=False)
    return out
